# revision 1
# baseline (speedup 1.0000x reference)
"""Deformable conv (B=4, C=256, H=W=64, 3x3, groups=4) on 8 trn2 cores.

Sharding: core = (batch b, H-half). Each core computes out[b, :, 32h:32h+32, :].

Per-core pipeline:
  1. offset conv (PE, fp32 im2col): off [18, 2048] = w_off * x
  2. PE-transpose off -> pixels-on-partitions [128, 16, 18]
  3. elementwise (DVE): sampling positions, floor, bilinear corner weights
     cw [128, 16, 4, 9] fp16, gather indices idx [128, 16, 9] int16
  4. idx rewrap to the dma_gather 16-partition-wrapped replicated layout
     (2 DMAs through a DRAM scratch)
  5. per (tap k, pixel-half): dma_gather (DRAM source, transpose=False) of
     fp16 "quad" tokens (all 4 bilinear corners x 256 channels in one 2KB
     elem) -> gq [128 pixels, 8 ranks, 1024]
  6. combine corners (DVE tensor_tensor with [128,8,1]->[128,8,256]
     broadcast page-scalars) -> val_t [128 pix, 8, 256] fp16
  7. PE-transpose val_t back to channel-major val [9k][2cb][128, 2048] fp16
  8. grouped deform matmul (PE, fp16, block-diag weights) accumulating over
     k into PSUM [2][128, 2048] fp32 -> out [256, 2048] fp32
"""

import sys

for _p in ("/opt/trn_rl_repo",):
    if _p not in sys.path:
        sys.path.insert(0, _p)

import numpy as np
from contextlib import ExitStack

import concourse.bass as bass
import concourse.bacc as bacc
import concourse.tile as tile
import concourse.mybir as mybir

dt = mybir.dt
F32, F16, I16 = dt.float32, dt.float16, dt.int16
Alu = mybir.AluOpType

B, C, H, W = 4, 256, 64, 64
KH = KW = 3
K = 9
GROUPS, CG = 4, 64
HALF = 32                      # output rows per core
NPIX = HALF * W                # 2048 pixels per core
NT = NPIX // 128               # 16 pixel tiles of 128
WG = 66                       # virtual token grid width (cols -1..64)
NTOK = WG * WG                 # token (r, s) at (r+1)*66 + (s+1), r,s in [-1,64]
MAGIC = float(np.float32(1.5 * 2 ** 23))

_NC_CACHE = {}


def build_module():
    nc = bacc.Bacc(
        "TRN2", target_bir_lowering=False, debug=False, num_devices=8,
        num_swdge_queues=4,
    )

    # ---- DRAM I/O ----
    xpad0 = nc.dram_tensor("xpad0", [128, 34, 66], F16, kind="ExternalInput").ap()
    xpad1 = nc.dram_tensor("xpad1", [128, 34, 66], F16, kind="ExternalInput").ap()
    xq = nc.dram_tensor("xq", [NTOK, 1024], F16, kind="ExternalInput").ap()
    woff = nc.dram_tensor("woff", [128, 2, 9, 18], F16, kind="ExternalInput").ap()
    wdef = nc.dram_tensor("wdef", [128, 9, 2, 128], F16, kind="ExternalInput").ap()
    ybk = nc.dram_tensor("ybk", [128, 16, 9], F32, kind="ExternalInput").ap()
    xbk = nc.dram_tensor("xbk", [128, 16, 9], F32, kind="ExternalInput").ap()
    eye16_d = nc.dram_tensor("eye16", [128, 128], F16, kind="ExternalInput").ap()
    eye32_d = nc.dram_tensor("eye32", [128, 128], F32, kind="ExternalInput").ap()
    out_d = nc.dram_tensor("out", [256, NPIX], F32, kind="ExternalOutput").ap()
    # scratch for idx rewrap
    idx_scr = nc.dram_tensor("idx_scr", [16, 9 * 128], I16, kind="Internal").ap()

    with tile.TileContext(nc) as tc, ExitStack() as ctx:
        const = ctx.enter_context(tc.tile_pool(name="const", bufs=1))
        sb = ctx.enter_context(tc.tile_pool(name="sb", bufs=1))
        small = ctx.enter_context(tc.tile_pool(name="small", bufs=1))
        poolxp = tc.tile_pool(name="poolxp", bufs=1)
        pxp = poolxp.__enter__()

        # ---- load constants ----
        xp0_sb = pxp.tile([128, 34, 66], F16, tag="xp0", name="xp0")
        xp1_sb = pxp.tile([128, 34, 66], F16, tag="xp1", name="xp1")
        xp_sb = [xp0_sb, xp1_sb]
        nc.sync.dma_start(xp_sb[0][:], xpad0)
        nc.sync.dma_start(xp_sb[1][:], xpad1)
        woff_sb = const.tile([128, 2, 9, 18], F16)
        nc.sync.dma_start(woff_sb[:], woff)
        wdef_sb = const.tile([128, 9, 2, 128], F16)
        nc.sync.dma_start(wdef_sb[:], wdef)
        ybk_sb = const.tile([128, 16, 9], F32)
        nc.sync.dma_start(ybk_sb[:], ybk)
        xbk_sb = const.tile([128, 16, 9], F32)
        nc.sync.dma_start(xbk_sb[:], xbk)
        eye16 = const.tile([128, 128], F16)
        nc.sync.dma_start(eye16[:], eye16_d)
        eye32 = const.tile([128, 128], F32)
        nc.sync.dma_start(eye32[:], eye32_d)

        # =============== stage 1: offset conv ===============
        off_sb = sb.tile([18, NPIX], F16)
        with nc.named_scope("s1_offconv"), \
             tc.tile_pool(name="psA", bufs=1, space="PSUM") as psA:
            off_ps = psA.tile([18, NPIX], F32)
            for n in range(4):
                q = 0
                for k in range(9):
                    ki, kj = k // 3, k % 3
                    for cb in range(2):
                        rhs = xp_sb[cb][:, n * 8 + ki: n * 8 + ki + 8, kj: kj + 64]
                        nc.tensor.matmul(
                            off_ps[:, n * 512:(n + 1) * 512],
                            woff_sb[:, cb, k, :],
                            rhs,
                            start=(q == 0),
                            stop=(q == 17),
                        )
                        q += 1
            nc.scalar.copy(off_sb[:], off_ps[:])
        poolxp.__exit__(None, None, None)

        # =============== stage 2: transpose off -> [128, 16, 18] ===============
        offT = sb.tile([128, 16, 18], F16)
        with nc.named_scope("s2_offT"), \
             tc.tile_pool(name="psB", bufs=2, space="PSUM") as psB:
            for t in range(NT):
                ps_t = psB.tile([128, 18], F16, tag="offT", name="ps_t")
                nc.tensor.transpose(
                    ps_t[:], off_sb[:, t * 128:(t + 1) * 128], eye16[0:18, 0:18]
                )
                nc.scalar.copy(offT[:, t, :], ps_t[:])

        # =============== stage 3: positions / weights / indices ===============
        def t3(nm):
            return small.tile([128, 16, 9], F32, tag=nm, name=nm)

        cw = sb.tile([128, 16, 4, 9], F16)
        idxf = sb.tile([128, 16, 9], I16)

        def axis_weights(off_slice, grid, hi):
            """returns (floor, cw0, cw1) for one axis; hi = max valid coord"""
            p = t3("p")
            nc.vector.tensor_tensor(p[:], off_slice, grid[:], Alu.add)
            r = t3("r")
            nc.vector.tensor_scalar_add(r[:], p[:], MAGIC)
            nc.vector.tensor_scalar_sub(r[:], r[:], MAGIC)
            g = t3("g")
            nc.vector.tensor_tensor(g[:], r[:], p[:], Alu.is_gt)
            f = small.tile([128, 16, 9], F32, tag="keepf", name="keepf", bufs=2)
            nc.vector.tensor_sub(f[:], r[:], g[:])
            w1 = t3("w1")
            nc.vector.tensor_sub(w1[:], p[:], f[:])
            w0 = t3("w0")
            nc.vector.tensor_scalar(w0[:], w1[:], -1.0, 1.0, Alu.mult, Alu.add)
            a = t3("a")
            b = t3("b")
            v0 = t3("v0")
            nc.vector.tensor_scalar(a[:], f[:], 0.0, None, Alu.is_ge)
            nc.vector.tensor_scalar(b[:], f[:], float(hi), None, Alu.is_le)
            nc.vector.tensor_tensor(v0[:], a[:], b[:], Alu.mult)
            v1 = t3("v1")
            nc.vector.tensor_scalar(a[:], f[:], -1.0, None, Alu.is_ge)
            nc.vector.tensor_scalar(b[:], f[:], float(hi - 1), None, Alu.is_le)
            nc.vector.tensor_tensor(v1[:], a[:], b[:], Alu.mult)
            cw0 = small.tile([128, 16, 9], F32, tag="keep0", name="keep0", bufs=2)
            cw1 = small.tile([128, 16, 9], F32, tag="keep1", name="keep1", bufs=2)
            nc.vector.tensor_tensor(cw0[:], w0[:], v0[:], Alu.mult)
            nc.vector.tensor_tensor(cw1[:], w1[:], v1[:], Alu.mult)
            return f, cw0, cw1

        fy, cwy0, cwy1 = axis_weights(offT[:, :, 0:18:2], ybk_sb, 63)
        fx, cwx0, cwx1 = axis_weights(offT[:, :, 1:18:2], xbk_sb, 63)

        # corner products -> cw[128, 16, corner(cy*2+cx), 9] fp16
        for cy, cwy in ((0, cwy0), (1, cwy1)):
            for cx, cwx in ((0, cwx0), (1, cwx1)):
                nc.vector.tensor_tensor(
                    cw[:, :, cy * 2 + cx, :], cwy[:], cwx[:], Alu.mult
                )

        # indices: flat = clip(fy,-1,64)*64 + clip(fx,-1,63) + PADT
        r_ = t3("r_")
        nc.vector.tensor_scalar(r_[:], fy[:], -1.0, 64.0, Alu.max, Alu.min)
        s_ = t3("s_")
        nc.vector.tensor_scalar(s_[:], fx[:], -1.0, 63.0, Alu.max, Alu.min)
        fl = t3("fl")
        nc.vector.tensor_scalar(fl[:], r_[:], 66.0, None, Alu.mult)
        nc.vector.tensor_tensor(fl[:], fl[:], s_[:], Alu.add)
        nc.vector.tensor_scalar_add(fl[:], fl[:], 67.0)
        nc.vector.tensor_copy(idxf[:], fl[:])

        # =============== stage 4: idx rewrap + replicate ===============
        # SBUF [128 q, 16 t, 9 k] -> DRAM [16, 9k*128] at (q%16, k*128 + t*8 + q//16)
        idxw = sb.tile([128, 9 * 128], I16)
        idxs2 = sb.tile([16, 8, 16, 9], I16)  # [p, m(q//16), t, k]
        # 8 partition-move DMAs (contiguous free both sides)
        for m in range(8):
            nc.sync.dma_start(idxs2[:, m, :, :], idxf[16 * m:16 * (m + 1), :, :])
        # DVE free-dim permute: idxw[0:16, k*128 + t*8 + m] = idxs2[p, m, t, k]
        s2 = idxs2[:]
        src_ap = bass.AP(
            s2.tensor, s2.offset,
            [list(s2.ap[0]), [1, 9], [9, 16], [144, 8]],  # p; k, t, m
        )
        ow = idxw[0:16, :]
        dst_ap = bass.AP(
            ow.tensor, ow.offset,
            [list(ow.ap[0]), [128, 9], [8, 16], [1, 8]],
        )
        nc.vector.tensor_copy(dst_ap, src_ap)
        # replicate partitions 0:16 -> all 128 (3 doublings)
        for step in (16, 32, 64):
            nc.sync.dma_start(idxw[step:2 * step, :], idxw[0:step, :])

        # =============== stages 5-7: gather / combine / transpose ===============
        gpool = ctx.enter_context(tc.tile_pool(name="gq", bufs=3))
        vpool = ctx.enter_context(tc.tile_pool(name="vt", bufs=2))
        val = sb.tile([128, 9, 2, NPIX], F16)   # channel-major val, all taps

        with nc.named_scope("s57_gcx"), \
             tc.tile_pool(name="psV", bufs=4, space="PSUM") as psV:
            for k in range(9):
                for h2 in range(2):
                    gq = gpool.tile([128, 8, 1024], F16, tag="gq", name="gq")
                    nc.gpsimd.dma_gather(
                        gq[:],
                        xq,
                        idxw[:, k * 128 + h2 * 64: k * 128 + h2 * 64 + 64],
                        1024,
                        1024,
                        1024,
                        transpose=False,
                        queue_num=(k * 2 + h2) % 4,
                        single_packet=False,
                    )
                    # weight the 4 corner slices, combine on DVE
                    m0 = vpool.tile([128, 8, 256], F16, tag="m0", name="m0")
                    m1 = vpool.tile([128, 8, 256], F16, tag="m1", name="m1")
                    m2 = vpool.tile([128, 8, 256], F16, tag="m2", name="m2")
                    m3 = vpool.tile([128, 8, 256], F16, tag="m3", name="m3")
                    cws = cw[:, h2 * 8:(h2 + 1) * 8, :, :]

                    def cwb(corner):
                        return cws[:, :, corner, k:k + 1].broadcast_to([128, 8, 256])

                    # elem layout: [cx(2), cy(2), c(256)]; corner idx = cy*2+cx
                    nc.vector.tensor_tensor(
                        m0[:], gq[:, :, 0:256], cwb(0), Alu.mult)
                    nc.vector.tensor_tensor(
                        m2[:], gq[:, :, 256:512], cwb(2), Alu.mult)
                    nc.vector.tensor_tensor(
                        m1[:], gq[:, :, 512:768], cwb(1), Alu.mult)
                    nc.vector.tensor_tensor(
                        m3[:], gq[:, :, 768:1024], cwb(3), Alu.mult)
                    nc.vector.tensor_add(m0[:], m0[:], m1[:])
                    nc.vector.tensor_add(m2[:], m2[:], m3[:])
                    nc.vector.tensor_add(m0[:], m0[:], m2[:])

                    # transpose to channel-major
                    for cb in range(2):
                        for rg in range(2):
                            ps_v = psV.tile([128, 512], F16, tag="psv", name="psv")
                            for rr in range(4):
                                rank = rg * 4 + rr
                                nc.tensor.transpose(
                                    ps_v[:, rr * 128:(rr + 1) * 128],
                                    m0[:, rank, cb * 128:(cb + 1) * 128],
                                    eye16[:],
                                )
                            nc.scalar.copy(
                                val[:, k, cb,
                                    h2 * 1024 + rg * 512: h2 * 1024 + (rg + 1) * 512],
                                ps_v[:],
                            )

        # =============== stage 8: deform matmul + output ===============
        outsb = sb.tile([128, 2, NPIX], F32)
        with nc.named_scope("s8_matmul"), \
             tc.tile_pool(name="psD", bufs=1, space="PSUM") as psD:
            for cb in range(2):
                ps_o = psD.tile([128, NPIX], F32, tag=f"psd{cb}", name=f"psd{cb}")
                for n in range(4):
                    for k in range(9):
                        nc.tensor.matmul(
                            ps_o[:, n * 512:(n + 1) * 512],
                            wdef_sb[:, k, cb, :],
                            val[:, k, cb, n * 512:(n + 1) * 512],
                            start=(k == 0),
                            stop=(k == 8),
                        )
                nc.scalar.copy(outsb[:, cb, :], ps_o[:])
                nc.sync.dma_start(out_d[cb * 128:(cb + 1) * 128, :], outsb[:, cb, :])

    nc.compile()
    return nc


def prep_inputs(x, w_off, b_off, w_deform):
    """host-side layout prep; returns list of 8 in_maps"""
    x = np.asarray(x, np.float32)
    w_off = np.asarray(w_off, np.float32)
    b_off = np.asarray(b_off, np.float32)
    w_deform = np.asarray(w_deform, np.float32)

    # weights (shared by all cores)
    # woff[ch, cb, k, oc] = w_off[oc, cb*128+ch, k//3, k%3]
    wo = w_off.reshape(18, 2, 128, 3, 3).transpose(2, 1, 3, 4, 0)  # ch, cb, ki, kj, oc
    woff = np.ascontiguousarray(wo.reshape(128, 2, 9, 18), np.float16)
    # wdef[row, k, cb, col]: block-diag of groups (2cb, 2cb+1)
    wd = np.zeros((128, 9, 2, 128), np.float32)
    wg = w_deform.reshape(4, 64, 64, 9)  # [g, o, i, k]
    for cb in range(2):
        for gg in range(2):
            g = 2 * cb + gg
            # rows gg*64..gg*64+63 = i, cols gg*64.. = o
            # target block shape [64 i, 9 k, 64 o]; wg[g] is [o, i, k]
            wd[gg * 64:(gg + 1) * 64, :, cb, gg * 64:(gg + 1) * 64] = (
                wg[g].transpose(1, 2, 0)
            )
    wdef = np.ascontiguousarray(wd, np.float16)

    eye16 = np.eye(128, dtype=np.float16)
    eye32 = np.eye(128, dtype=np.float32)

    # per-batch quad tokens
    xqs = []
    for b in range(B):
        P4 = np.pad(x[b], ((0, 0), (1, 2), (1, 2)))  # [256, 67, 67]
        xq = np.zeros((NTOK, 2, 2, 256), np.float16)
        for cx in range(2):
            for cy in range(2):
                sub = P4[:, cy:cy + WG, cx:cx + WG]          # [256, 66, 66]
                xq[:, cx, cy, :] = sub.transpose(1, 2, 0).reshape(NTOK, 256)
        xqs.append(np.ascontiguousarray(xq.reshape(NTOK, 1024)))

    in_maps = []
    for core in range(8):
        b, half = core // 2, core % 2
        yr0 = HALF * half
        # xpad [2][128, 34, 66]: rows yr0-1..yr0+32, 1 col pad each side
        xp = np.zeros((2, 128, 34, 66), np.float16)
        r0, r1 = yr0 - 1, yr0 + 33
        sr0, sr1 = max(r0, 0), min(r1, H)
        xp[:, :, sr0 - r0:sr0 - r0 + (sr1 - sr0), 1:65] = (
            x[b].reshape(2, 128, H, W)[:, :, sr0:sr1, :]
        )
        # grids
        q = np.arange(128)[:, None, None]
        t = np.arange(16)[None, :, None]
        k = np.arange(9)[None, None, :]
        pix = t * 128 + q
        ybk = (yr0 + pix // 64 + k // 3 - 1 + b_off[2 * k]).astype(np.float32)
        xbk = (pix % 64 + k % 3 - 1 + b_off[2 * k + 1]).astype(np.float32)
        in_maps.append({
            "xpad0": np.ascontiguousarray(xp[0]),
            "xpad1": np.ascontiguousarray(xp[1]),
            "xq": xqs[b],
            "woff": woff,
            "wdef": wdef,
            "ybk": np.ascontiguousarray(ybk),
            "xbk": np.ascontiguousarray(xbk),
            "eye16": eye16,
            "eye32": eye32,
        })
    return in_maps


def kernel(x, w_off, b_off, w_deform):
    from concourse.bass_utils import run_bass_kernel_spmd

    if "nc" not in _NC_CACHE:
        _NC_CACHE["nc"] = build_module()
    nc = _NC_CACHE["nc"]
    in_maps = prep_inputs(x, w_off, b_off, w_deform)
    res = run_bass_kernel_spmd(nc, in_maps, list(range(8)))
    out = np.zeros((B, C, H, W), np.float32)
    for core in range(8):
        b, half = core // 2, core % 2
        out[b, :, HALF * half:HALF * (half + 1), :] = (
            res.results[core]["out"].reshape(C, HALF, W)
        )
    return out



# revision 8
# speedup vs baseline: 1.5377x; 1.5377x over previous
"""Deformable conv (B=4, C=256, H=W=64, 3x3, groups=4) on 8 trn2 cores.

Sharding: core = (batch b, H-half). Each core computes out[b, :, 32h:32h+32, :].

Per-core pipeline:
  1. offset conv (PE, im2col): off [18, 2048] = w_off * x
  2. PE-transpose off -> pixels-on-partitions [128, 16, 18]
  3. elementwise (DVE): sampling positions, floor, bilinear corner weights
     cwd [128, 16, 4, 9, 2] fp16 (weights duplicated in adjacent pairs so the
     broadcast multiply AP has an inner step-1 dim -> DVE 2x mode),
     gather indices idx [128, 16, 9] int16
  4. idx rewrap to the dma_gather 16-partition-wrapped replicated layout
  5. main loop h2(2) x k(9):
     a. dma_gather fp16 quad tokens (4 bilinear corners x 256 ch per 2KB elem)
        -> gq [128 pix, 8 ranks, 1024]
     b. DVE: 4 corner-weight mults + 2 adds -> mA, mB [128, 8, 256]
     c. PE: transpose-accumulate T(mA)+T(mB) into PSUM -> valk [128, 2, 1024]
        channel-major (ACT copies PSUM->SBUF)
     d. PE: grouped deform matmul accumulating over k into ps_o [128,2,1024]
  6. per h2: copy ps_o -> out rows, DMA out
"""

import sys

for _p in ("/opt/trn_rl_repo",):
    if _p not in sys.path:
        sys.path.insert(0, _p)

import numpy as np
from contextlib import ExitStack

import concourse.bass as bass
import concourse.bacc as bacc
import concourse.tile as tile
import concourse.mybir as mybir

dt = mybir.dt
F32, F16, I16 = dt.float32, dt.float16, dt.int16
Alu = mybir.AluOpType

B, C, H, W = 4, 256, 64, 64
KH = KW = 3
K = 9
GROUPS, CG = 4, 64
HALF = 32                      # output rows per core
NPIX = HALF * W                # 2048 pixels per core
NT = NPIX // 128               # 16 pixel tiles of 128
WG = 66                        # virtual token grid width (cols -1..64)
NTOK = WG * WG                 # token (r, s) at (r+1)*66 + (s+1), r,s in [-1,64]
MAGIC = float(np.float32(1.5 * 2 ** 23))

_NC_CACHE = {}


def build_module():
    nc = bacc.Bacc(
        "TRN2", target_bir_lowering=False, debug=False, num_devices=8,
        num_swdge_queues=4,
    )

    # ---- DRAM I/O ----
    xpad0 = nc.dram_tensor("xpad0", [128, 34, 66], F16, kind="ExternalInput").ap()
    xpad1 = nc.dram_tensor("xpad1", [128, 34, 66], F16, kind="ExternalInput").ap()
    xq = nc.dram_tensor("xq", [NTOK, 1024], F16, kind="ExternalInput").ap()
    woff = nc.dram_tensor("woff", [128, 2, 9, 18], F16, kind="ExternalInput").ap()
    wdef = nc.dram_tensor("wdef", [128, 9, 2, 128], F16, kind="ExternalInput").ap()
    ybk = nc.dram_tensor("ybk", [128, 16, 9], F32, kind="ExternalInput").ap()
    xbk = nc.dram_tensor("xbk", [128, 16, 9], F32, kind="ExternalInput").ap()
    eye16_d = nc.dram_tensor("eye16", [128, 128], F16, kind="ExternalInput").ap()
    out_d = nc.dram_tensor("out", [256, NPIX], F32, kind="ExternalOutput").ap()
    # scratch for idx rewrap
    idx_scr = nc.dram_tensor("idx_scr", [16, 9 * 128], I16, kind="Internal").ap()

    with tile.TileContext(nc) as tc, ExitStack() as ctx:
        const = ctx.enter_context(tc.tile_pool(name="const", bufs=1))
        sb = ctx.enter_context(tc.tile_pool(name="sb", bufs=1))
        small = ctx.enter_context(tc.tile_pool(name="small", bufs=1))
        poolxp = tc.tile_pool(name="poolxp", bufs=1)
        pxp = poolxp.__enter__()

        # ---- load constants ----
        xp0_sb = pxp.tile([128, 34, 66], F16, tag="xp0", name="xp0")
        xp1_sb = pxp.tile([128, 34, 66], F16, tag="xp1", name="xp1")
        xp_sb = [xp0_sb, xp1_sb]
        nc.sync.dma_start(xp_sb[0][:], xpad0)
        nc.sync.dma_start(xp_sb[1][:], xpad1)
        woff_sb = const.tile([128, 2, 9, 18], F16)
        nc.sync.dma_start(woff_sb[:], woff)
        wdef_sb = const.tile([128, 9, 2, 128], F16)
        nc.sync.dma_start(wdef_sb[:], wdef)
        ybk_sb = const.tile([128, 16, 9], F32)
        nc.sync.dma_start(ybk_sb[:], ybk)
        xbk_sb = const.tile([128, 16, 9], F32)
        nc.sync.dma_start(xbk_sb[:], xbk)
        eye16 = const.tile([128, 128], F16)
        nc.sync.dma_start(eye16[:], eye16_d)

        # =============== stage 1: offset conv ===============
        off_sb = sb.tile([18, NPIX], F16)
        with nc.named_scope("s1_offconv"), \
             tc.tile_pool(name="psA", bufs=1, space="PSUM") as psA:
            off_ps = psA.tile([18, NPIX], F32)
            for n in range(4):
                q = 0
                for k in range(9):
                    ki, kj = k // 3, k % 3
                    for cb in range(2):
                        rhs = xp_sb[cb][:, n * 8 + ki: n * 8 + ki + 8,
                                        kj: kj + 64]
                        nc.tensor.matmul(
                            off_ps[:, n * 512:(n + 1) * 512],
                            woff_sb[:, cb, k, :],
                            rhs,
                            start=(q == 0),
                            stop=(q == 17),
                        )
                        q += 1
            nc.scalar.copy(off_sb[:], off_ps[:])
        poolxp.__exit__(None, None, None)

        # =============== stage 2: transpose off -> [128, 16, 18] ===============
        offT = sb.tile([128, 16, 18], F16)
        with nc.named_scope("s2_offT"), \
             tc.tile_pool(name="psB", bufs=2, space="PSUM") as psB:
            for t in range(NT):
                ps_t = psB.tile([128, 18], F16, tag="offT", name="ps_t")
                nc.tensor.transpose(
                    ps_t[:], off_sb[:, t * 128:(t + 1) * 128], eye16[0:18, 0:18]
                )
                nc.scalar.copy(offT[:, t, :], ps_t[:])

        # =============== stage 3: positions / weights / indices ===============
        def t3(nm):
            return small.tile([128, 16, 9], F32, tag=nm, name=nm)

        # corner weights, duplicated in adjacent pairs: cwd[..., corner, k, 0:2]
        cwd = sb.tile([128, 16, 4, 9, 2], F16)
        idxf = sb.tile([128, 16, 9], I16)

        def axis_weights(off_slice, grid, hi):
            """returns (floor, cw0, cw1) for one axis; hi = max valid coord"""
            p = t3("p")
            nc.vector.tensor_tensor(p[:], off_slice, grid[:], Alu.add)
            r = t3("r")
            nc.vector.tensor_scalar_add(r[:], p[:], MAGIC)
            nc.vector.tensor_scalar_sub(r[:], r[:], MAGIC)
            g = t3("g")
            nc.vector.tensor_tensor(g[:], r[:], p[:], Alu.is_gt)
            f = small.tile([128, 16, 9], F32, tag="keepf", name="keepf", bufs=2)
            nc.vector.tensor_sub(f[:], r[:], g[:])
            w1 = t3("w1")
            nc.vector.tensor_sub(w1[:], p[:], f[:])
            w0 = t3("w0")
            nc.vector.tensor_scalar(w0[:], w1[:], -1.0, 1.0, Alu.mult, Alu.add)
            a = t3("a")
            b = t3("b")
            v0 = t3("v0")
            nc.vector.tensor_scalar(a[:], f[:], 0.0, None, Alu.is_ge)
            nc.vector.tensor_scalar(b[:], f[:], float(hi), None, Alu.is_le)
            nc.vector.tensor_tensor(v0[:], a[:], b[:], Alu.mult)
            v1 = t3("v1")
            nc.vector.tensor_scalar(a[:], f[:], -1.0, None, Alu.is_ge)
            nc.vector.tensor_scalar(b[:], f[:], float(hi - 1), None, Alu.is_le)
            nc.vector.tensor_tensor(v1[:], a[:], b[:], Alu.mult)
            cw0 = small.tile([128, 16, 9], F32, tag="keep0", name="keep0", bufs=2)
            cw1 = small.tile([128, 16, 9], F32, tag="keep1", name="keep1", bufs=2)
            nc.vector.tensor_tensor(cw0[:], w0[:], v0[:], Alu.mult)
            nc.vector.tensor_tensor(cw1[:], w1[:], v1[:], Alu.mult)
            return f, cw0, cw1

        fy, cwy0, cwy1 = axis_weights(offT[:, :, 0:18:2], ybk_sb, 63)
        fx, cwx0, cwx1 = axis_weights(offT[:, :, 1:18:2], xbk_sb, 63)

        # corner products -> cwd[128, 16, corner(cy*2+cx), 9, 0:2] fp16 (dup pair)
        for cy, cwy in ((0, cwy0), (1, cwy1)):
            for cx, cwx in ((0, cwx0), (1, cwx1)):
                for d in range(2):
                    nc.vector.tensor_tensor(
                        cwd[:, :, cy * 2 + cx, :, d], cwy[:], cwx[:], Alu.mult
                    )

        # indices: flat = clip(fy,-1,64)*64 + clip(fx,-1,63) + PADT
        r_ = t3("r_")
        nc.vector.tensor_scalar(r_[:], fy[:], -1.0, 64.0, Alu.max, Alu.min)
        s_ = t3("s_")
        nc.vector.tensor_scalar(s_[:], fx[:], -1.0, 63.0, Alu.max, Alu.min)
        fl = t3("fl")
        nc.vector.tensor_scalar(fl[:], r_[:], 66.0, None, Alu.mult)
        nc.vector.tensor_tensor(fl[:], fl[:], s_[:], Alu.add)
        nc.vector.tensor_scalar_add(fl[:], fl[:], 67.0)
        nc.vector.tensor_copy(idxf[:], fl[:])

        # =============== stage 4: idx rewrap + replicate ===============
        # SBUF [128 q, 16 t, 9 k] -> DRAM [16, 9k*128] at (q%16, k*128 + t*8 + q//16)
        idxw = sb.tile([128, 9 * 128], I16)
        idxs2 = sb.tile([16, 8, 16, 9], I16)  # [p, m(q//16), t, k]
        # 8 partition-move DMAs (contiguous free both sides)
        for m in range(8):
            nc.sync.dma_start(idxs2[:, m, :, :], idxf[16 * m:16 * (m + 1), :, :])
        # DVE free-dim permute: idxw[0:16, k*128 + t*8 + m] = idxs2[p, m, t, k]
        s2 = idxs2[:]
        src_ap = bass.AP(
            s2.tensor, s2.offset,
            [list(s2.ap[0]), [1, 9], [9, 16], [144, 8]],  # p; k, t, m
        )
        ow = idxw[0:16, :]
        dst_ap = bass.AP(
            ow.tensor, ow.offset,
            [list(ow.ap[0]), [128, 9], [8, 16], [1, 8]],
        )
        nc.vector.tensor_copy(dst_ap, src_ap)
        # replicate partitions 0:16 -> all 128 (3 doublings)
        for step in (16, 32, 64):
            nc.sync.dma_start(idxw[step:2 * step, :], idxw[0:step, :])

        # =============== stages 5-8: gather / combine / transpose / matmul ===
        gpool = ctx.enter_context(tc.tile_pool(name="gq", bufs=3))
        mpool = ctx.enter_context(tc.tile_pool(name="mt", bufs=2))
        vpool = ctx.enter_context(tc.tile_pool(name="vt", bufs=2))
        opool = ctx.enter_context(tc.tile_pool(name="ot", bufs=2))

        cwd_t = cwd.tensor

        def cwb2(h2, corner, k):
            """broadcast AP over cwd: [128, 8 ranks, 256] with inner step-1
            pair (w, w) repeated 128x: dims (rank: 72, rep: 0 x128, dup: 1 x2)"""
            base = cwd[:]
            off = base.offset + ((h2 * 8) * 72 + corner * 18 + k * 2)
            return bass.AP(
                base.tensor, off,
                [list(base.ap[0]), [72, 8], [0, 128], [1, 2]],
            )

        with nc.named_scope("s58_main"), \
             tc.tile_pool(name="psV", bufs=3, space="PSUM") as psV, \
             tc.tile_pool(name="psD", bufs=1, space="PSUM") as psD:
            for h2 in range(2):
                ps_o = psD.tile([128, 2, 1024], F32, tag="pso", name="pso")
                for k in range(9):
                    gq = gpool.tile([128, 8, 1024], F16, tag="gq", name="gq")
                    nc.gpsimd.dma_gather(
                        gq[:],
                        xq,
                        idxw[:, k * 128 + h2 * 64: k * 128 + h2 * 64 + 64],
                        1024,
                        1024,
                        1024,
                        transpose=False,
                        queue_num=(k + h2 * 2) % 4,
                        single_packet=False,
                    )
                    # weight the 4 corner slices on DVE (2x mode via dup pairs)
                    mA = mpool.tile([128, 8, 256], F16, tag="mA", name="mA")
                    mB = mpool.tile([128, 8, 256], F16, tag="mB", name="mB")
                    tA = mpool.tile([128, 8, 256], F16, tag="tA", name="tA")
                    tB = mpool.tile([128, 8, 256], F16, tag="tB", name="tB")
                    # elem layout: [cx(2), cy(2), c(256)]; corner idx = cy*2+cx
                    nc.vector.tensor_tensor(
                        mA[:], gq[:, :, 0:256], cwb2(h2, 0, k), Alu.mult)
                    nc.vector.tensor_tensor(
                        tA[:], gq[:, :, 256:512], cwb2(h2, 2, k), Alu.mult)
                    nc.vector.tensor_tensor(
                        mB[:], gq[:, :, 512:768], cwb2(h2, 1, k), Alu.mult)
                    nc.vector.tensor_tensor(
                        tB[:], gq[:, :, 768:1024], cwb2(h2, 3, k), Alu.mult)
                    nc.vector.tensor_add(mA[:], mA[:], tA[:])
                    nc.vector.tensor_add(mB[:], mB[:], tB[:])

                    # transpose-accumulate to channel-major
                    valk = vpool.tile([128, 2, 1024], F16, tag="valk",
                                      name="valk")
                    for cb in range(2):
                        for rg in range(2):
                            ps_v = psV.tile([128, 512], F32, tag="psv",
                                            name="psv")
                            for rr in range(4):
                                rank = rg * 4 + rr
                                nc.tensor.matmul(
                                    ps_v[:, rr * 128:(rr + 1) * 128],
                                    mA[:, rank, cb * 128:(cb + 1) * 128],
                                    eye16[:],
                                    start=True, stop=False,
                                )
                                nc.tensor.matmul(
                                    ps_v[:, rr * 128:(rr + 1) * 128],
                                    mB[:, rank, cb * 128:(cb + 1) * 128],
                                    eye16[:],
                                    start=False, stop=True,
                                )
                            nc.scalar.copy(
                                valk[:, cb, rg * 512:(rg + 1) * 512], ps_v[:])

                    # deform matmul: accumulate over k into ps_o
                    for cb in range(2):
                        for nn in range(2):
                            nc.tensor.matmul(
                                ps_o[:, cb, nn * 512:(nn + 1) * 512],
                                wdef_sb[:, k, cb, :],
                                valk[:, cb, nn * 512:(nn + 1) * 512],
                                start=(k == 0),
                                stop=(k == 8),
                            )

                # epilogue for this h2: PSUM -> SBUF -> DRAM
                outsb = opool.tile([128, 2, 1024], F32, tag="outsb",
                                   name="outsb")
                for cb in range(2):
                    nc.scalar.copy(outsb[:, cb, :], ps_o[:, cb, :])
                    nc.sync.dma_start(
                        out_d[cb * 128:(cb + 1) * 128,
                              h2 * 1024:(h2 + 1) * 1024],
                        outsb[:, cb, :],
                    )

    nc.compile()
    return nc


def prep_inputs(x, w_off, b_off, w_deform):
    """host-side layout prep; returns list of 8 in_maps"""
    x = np.asarray(x, np.float32)
    w_off = np.asarray(w_off, np.float32)
    b_off = np.asarray(b_off, np.float32)
    w_deform = np.asarray(w_deform, np.float32)

    # weights (shared by all cores)
    # woff[ch, cb, k, oc] = w_off[oc, cb*128+ch, k//3, k%3]
    wo = w_off.reshape(18, 2, 128, 3, 3).transpose(2, 1, 3, 4, 0)  # ch, cb, ki, kj, oc
    woff = np.ascontiguousarray(wo.reshape(128, 2, 9, 18), np.float16)
    # wdef[row, k, cb, col]: block-diag of groups (2cb, 2cb+1)
    wd = np.zeros((128, 9, 2, 128), np.float32)
    wg = w_deform.reshape(4, 64, 64, 9)  # [g, o, i, k]
    for cb in range(2):
        for gg in range(2):
            g = 2 * cb + gg
            # rows gg*64..gg*64+63 = i, cols gg*64.. = o
            # target block shape [64 i, 9 k, 64 o]; wg[g] is [o, i, k]
            wd[gg * 64:(gg + 1) * 64, :, cb, gg * 64:(gg + 1) * 64] = (
                wg[g].transpose(1, 2, 0)
            )
    wdef = np.ascontiguousarray(wd, np.float16)

    eye16 = np.eye(128, dtype=np.float16)

    # per-batch quad tokens
    xqs = []
    for b in range(B):
        P4 = np.pad(x[b], ((0, 0), (1, 2), (1, 2)))  # [256, 67, 67]
        xq = np.zeros((NTOK, 2, 2, 256), np.float16)
        for cx in range(2):
            for cy in range(2):
                sub = P4[:, cy:cy + WG, cx:cx + WG]          # [256, 66, 66]
                xq[:, cx, cy, :] = sub.transpose(1, 2, 0).reshape(NTOK, 256)
        xqs.append(np.ascontiguousarray(xq.reshape(NTOK, 1024)))

    in_maps = []
    for core in range(8):
        b, half = core // 2, core % 2
        yr0 = HALF * half
        # xpad [2][128, 34, 66]: rows yr0-1..yr0+32, 1 col pad each side
        xp = np.zeros((2, 128, 34, 66), np.float16)
        r0, r1 = yr0 - 1, yr0 + 33
        sr0, sr1 = max(r0, 0), min(r1, H)
        xp[:, :, sr0 - r0:sr0 - r0 + (sr1 - sr0), 1:65] = (
            x[b].reshape(2, 128, H, W)[:, :, sr0:sr1, :]
        )
        # grids
        q = np.arange(128)[:, None, None]
        t = np.arange(16)[None, :, None]
        k = np.arange(9)[None, None, :]
        pix = t * 128 + q
        ybk = (yr0 + pix // 64 + k // 3 - 1 + b_off[2 * k]).astype(np.float32)
        xbk = (pix % 64 + k % 3 - 1 + b_off[2 * k + 1]).astype(np.float32)
        in_maps.append({
            "xpad0": np.ascontiguousarray(xp[0]),
            "xpad1": np.ascontiguousarray(xp[1]),
            "xq": xqs[b],
            "woff": woff,
            "wdef": wdef,
            "ybk": np.ascontiguousarray(ybk),
            "xbk": np.ascontiguousarray(xbk),
            "eye16": eye16,
        })
    return in_maps


def kernel(x, w_off, b_off, w_deform):
    from concourse.bass_utils import run_bass_kernel_spmd

    if "nc" not in _NC_CACHE:
        _NC_CACHE["nc"] = build_module()
    nc = _NC_CACHE["nc"]
    in_maps = prep_inputs(x, w_off, b_off, w_deform)
    res = run_bass_kernel_spmd(nc, in_maps, list(range(8)))
    out = np.zeros((B, C, H, W), np.float32)
    for core in range(8):
        b, half = core // 2, core % 2
        out[b, :, HALF * half:HALF * (half + 1), :] = (
            res.results[core]["out"].reshape(C, HALF, W)
        )
    return out


# revision 13
# speedup vs baseline: 1.8395x; 1.1963x over previous
"""Deformable conv (B=4, C=256, H=W=64, 3x3, groups=4) on 8 trn2 cores.

Sharding: core = (batch b, H-half). Each core computes out[b, :, 32h:32h+32, :].

Per-core pipeline (prologue split by output half hh so gathers start early):
  per half hh (pixels hh*1024..+1024, t-tiles hh*8..+8):
    1. offset conv (PE, im2col): off{hh} [18, 1024] = w_off * x
    2. PE-transpose -> offT{hh} [128, 8, 18] (pixels on partitions)
    3. DVE: sampling positions, floor, bilinear corner weights
       cwd{hh} [128, 8, 4, 9, 2] fp16 (weights duplicated in adjacent pairs so
       the broadcast multiply AP has an inner step-1 dim -> DVE 2x mode),
       gather indices idxf{hh} [128, 8, 9] int16
    4. idx rewrap to dma_gather 16-partition-wrapped replicated layout
       idxw{hh} [128, 9*64]
  main loop h2(2) x k(9):
    5. dma_gather fp16 quad tokens (4 bilinear corners x 256 ch per 2KB elem)
       -> gq [128 pix, 8 ranks, 1024]
    6. DVE: 4 corner-weight mults -> m0..m3 [128, 8, 256]
    7. PE: transpose-accumulate T(m0)+..+T(m3) into PSUM (channel-major),
       ACT copies PSUM -> valk [128, 2, 1024]
    8. PE: grouped deform matmul accumulating over k into ps_o [128,2,1024]
  per h2: copy ps_o -> out rows, DMA out
"""

import sys

for _p in ("/opt/trn_rl_repo",):
    if _p not in sys.path:
        sys.path.insert(0, _p)

import numpy as np
from contextlib import ExitStack

import concourse.bass as bass
import concourse.bacc as bacc
import concourse.tile as tile
import concourse.mybir as mybir

dt = mybir.dt
F32, F16, I16 = dt.float32, dt.float16, dt.int16
Alu = mybir.AluOpType

B, C, H, W = 4, 256, 64, 64
KH = KW = 3
K = 9
GROUPS, CG = 4, 64
HALF = 32                      # output rows per core
NPIX = HALF * W                # 2048 pixels per core
NT = NPIX // 128               # 16 pixel tiles of 128
WG = 66                        # virtual token grid width (cols -1..64)
NTOK = WG * WG                 # token (r, s) at (r+1)*66 + (s+1), r,s in [-1,64]
MAGIC = float(np.float32(1.5 * 2 ** 23))

_NC_CACHE = {}


def build_module():
    nc = bacc.Bacc(
        "TRN2", target_bir_lowering=False, debug=False, num_devices=8,
        num_swdge_queues=4,
    )

    # ---- DRAM I/O ----
    xpad0 = nc.dram_tensor("xpad0", [128, 34, 66], F16, kind="ExternalInput").ap()
    xpad1 = nc.dram_tensor("xpad1", [128, 34, 66], F16, kind="ExternalInput").ap()
    xq = nc.dram_tensor("xq", [NTOK, 1024], F16, kind="ExternalInput").ap()
    woff = nc.dram_tensor("woff", [128, 2, 9, 18], F16, kind="ExternalInput").ap()
    wdef = nc.dram_tensor("wdef", [128, 9, 2, 128], F16, kind="ExternalInput").ap()
    ybk = nc.dram_tensor("ybk", [128, 16, 9], F32, kind="ExternalInput").ap()
    xbk = nc.dram_tensor("xbk", [128, 16, 9], F32, kind="ExternalInput").ap()
    eye16_d = nc.dram_tensor("eye16", [128, 128], F16, kind="ExternalInput").ap()
    out_d = nc.dram_tensor("out", [256, NPIX], F32, kind="ExternalOutput").ap()

    with tile.TileContext(nc) as tc, ExitStack() as ctx:
        const = ctx.enter_context(tc.tile_pool(name="const", bufs=1))
        sb = ctx.enter_context(tc.tile_pool(name="sb", bufs=1))
        small = ctx.enter_context(tc.tile_pool(name="small", bufs=1))
        poolxp = tc.tile_pool(name="poolxp", bufs=1)
        pxp = poolxp.__enter__()

        # ---- load constants ----
        xp0_sb = pxp.tile([128, 34, 66], F16, tag="xp0", name="xp0")
        xp1_sb = pxp.tile([128, 34, 66], F16, tag="xp1", name="xp1")
        xp_sb = [xp0_sb, xp1_sb]
        nc.sync.dma_start(xp_sb[0][:], xpad0)
        nc.sync.dma_start(xp_sb[1][:], xpad1)
        woff_sb = const.tile([128, 2, 9, 18], F16)
        nc.sync.dma_start(woff_sb[:], woff)
        wdef_sb = const.tile([128, 9, 2, 128], F16)
        nc.sync.dma_start(wdef_sb[:], wdef)
        ybk_sb = const.tile([128, 16, 9], F32)
        nc.sync.dma_start(ybk_sb[:], ybk)
        xbk_sb = const.tile([128, 16, 9], F32)
        nc.sync.dma_start(xbk_sb[:], xbk)
        eye16 = const.tile([128, 128], F16)
        nc.sync.dma_start(eye16[:], eye16_d)

        # per-half prologue outputs
        cwds = []
        idxws = []

        pool_psA = tc.tile_pool(name="psA", bufs=1, space="PSUM")
        pool_psB = tc.tile_pool(name="psB", bufs=2, space="PSUM")
        psA = pool_psA.__enter__()
        psB = pool_psB.__enter__()

        def t3(nm, hh):
            return small.tile([128, 8, 9], F32, tag=f"{nm}{hh}",
                              name=f"{nm}{hh}")

        def axis_weights(hh, off_slice, grid, hi):
            """returns (floor, cw0, cw1) for one axis; hi = max valid coord"""
            p = t3("p", hh)
            nc.vector.tensor_tensor(p[:], off_slice, grid, Alu.add)
            r = t3("r", hh)
            nc.vector.tensor_scalar_add(r[:], p[:], MAGIC)
            nc.vector.tensor_scalar_sub(r[:], r[:], MAGIC)
            g = t3("g", hh)
            nc.vector.tensor_tensor(g[:], r[:], p[:], Alu.is_gt)
            f = small.tile([128, 8, 9], F32, tag=f"keepf{hh}",
                           name=f"keepf{hh}", bufs=2)
            nc.vector.tensor_sub(f[:], r[:], g[:])
            w1 = t3("w1", hh)
            nc.vector.tensor_sub(w1[:], p[:], f[:])
            w0 = t3("w0", hh)
            nc.vector.tensor_scalar(w0[:], w1[:], -1.0, 1.0, Alu.mult, Alu.add)
            a = t3("a", hh)
            b = t3("b", hh)
            v0 = t3("v0", hh)
            nc.vector.tensor_scalar(a[:], f[:], 0.0, None, Alu.is_ge)
            nc.vector.tensor_scalar(b[:], f[:], float(hi), None, Alu.is_le)
            nc.vector.tensor_tensor(v0[:], a[:], b[:], Alu.mult)
            v1 = t3("v1", hh)
            nc.vector.tensor_scalar(a[:], f[:], -1.0, None, Alu.is_ge)
            nc.vector.tensor_scalar(b[:], f[:], float(hi - 1), None, Alu.is_le)
            nc.vector.tensor_tensor(v1[:], a[:], b[:], Alu.mult)
            cw0 = small.tile([128, 8, 9], F32, tag=f"keep0{hh}",
                             name=f"keep0{hh}", bufs=2)
            cw1 = small.tile([128, 8, 9], F32, tag=f"keep1{hh}",
                             name=f"keep1{hh}", bufs=2)
            nc.vector.tensor_tensor(cw0[:], w0[:], v0[:], Alu.mult)
            nc.vector.tensor_tensor(cw1[:], w1[:], v1[:], Alu.mult)
            return f, cw0, cw1

        for hh in range(2):
            tr = slice(hh * 8, hh * 8 + 8)
            # ---- s1: offset conv for this half -> off_h [18, 1024]
            off_h = sb.tile([18, 1024], F16, tag=f"off{hh}", name=f"off{hh}")
            with nc.named_scope(f"s1_offconv{hh}"):
                off_ps = psA.tile([18, 1024], F32, tag=f"offps{hh}",
                                  name=f"offps{hh}")
                for n2 in range(2):
                    n = hh * 2 + n2
                    q = 0
                    for k in range(9):
                        ki, kj = k // 3, k % 3
                        for cb in range(2):
                            rhs = xp_sb[cb][:, n * 8 + ki: n * 8 + ki + 8,
                                            kj: kj + 64]
                            nc.tensor.matmul(
                                off_ps[:, n2 * 512:(n2 + 1) * 512],
                                woff_sb[:, cb, k, :],
                                rhs,
                                start=(q == 0),
                                stop=(q == 17),
                            )
                            q += 1
                nc.scalar.copy(off_h[:], off_ps[:])

            # ---- s2: transpose -> offT_h [128, 8, 18]
            offT = sb.tile([128, 8, 18], F16, tag=f"offT{hh}", name=f"offT{hh}")
            with nc.named_scope(f"s2_offT{hh}"):
                for t8 in range(8):
                    ps_t = psB.tile([128, 18], F16, tag="offT", name="ps_t")
                    nc.tensor.transpose(
                        ps_t[:], off_h[:, t8 * 128:(t8 + 1) * 128],
                        eye16[0:18, 0:18]
                    )
                    nc.scalar.copy(offT[:, t8, :], ps_t[:])

            # ---- s3: weights + indices for this half
            cwd = sb.tile([128, 8, 4, 9, 2], F16, tag=f"cwd{hh}",
                          name=f"cwd{hh}")
            idxf = sb.tile([128, 8, 9], I16, tag=f"idxf{hh}", name=f"idxf{hh}")
            cwds.append(cwd)

            fy, cwy0, cwy1 = axis_weights(
                hh, offT[:, :, 0:18:2], ybk_sb[:, tr, :], 63)
            fx, cwx0, cwx1 = axis_weights(
                hh, offT[:, :, 1:18:2], xbk_sb[:, tr, :], 63)

            # corner products -> cwd[128, 8, corner(cy*2+cx), 9, 0:2] (dup)
            for cy, cwy in ((0, cwy0), (1, cwy1)):
                for cx, cwx in ((0, cwx0), (1, cwx1)):
                    for d in range(2):
                        nc.vector.tensor_tensor(
                            cwd[:, :, cy * 2 + cx, :, d], cwy[:], cwx[:],
                            Alu.mult
                        )

            # indices: flat = clip(fy,-1,64)*66 + clip(fx,-1,63) + 67
            r_ = t3("r_", hh)
            nc.vector.tensor_scalar(r_[:], fy[:], -1.0, 64.0, Alu.max, Alu.min)
            s_ = t3("s_", hh)
            nc.vector.tensor_scalar(s_[:], fx[:], -1.0, 63.0, Alu.max, Alu.min)
            fl = t3("fl", hh)
            nc.vector.tensor_scalar(fl[:], r_[:], 66.0, None, Alu.mult)
            nc.vector.tensor_tensor(fl[:], fl[:], s_[:], Alu.add)
            nc.vector.tensor_scalar_add(fl[:], fl[:], 67.0)
            nc.vector.tensor_copy(idxf[:], fl[:])

            # ---- s4: idx rewrap + replicate for this half
            # SBUF [128 q, 8 t, 9 k] -> idxw[q%16, k*64 + t*8 + q//16]
            idxw = sb.tile([128, 9 * 64], I16, tag=f"idxw{hh}",
                           name=f"idxw{hh}")
            idxs2 = sb.tile([16, 8, 8, 9], I16, tag=f"idxs2{hh}",
                            name=f"idxs2{hh}")  # [p, m(q//16), t8, k]
            idxws.append(idxw)
            for m in range(8):
                nc.sync.dma_start(idxs2[:, m, :, :],
                                  idxf[16 * m:16 * (m + 1), :, :])
            # DVE free-dim permute: idxw[0:16, k*64 + t*8 + m] = idxs2[p,m,t,k]
            s2ap = idxs2[:]
            src_ap = bass.AP(
                s2ap.tensor, s2ap.offset,
                [list(s2ap.ap[0]), [1, 9], [9, 8], [72, 8]],  # p; k, t8, m
            )
            ow = idxw[0:16, :]
            dst_ap = bass.AP(
                ow.tensor, ow.offset,
                [list(ow.ap[0]), [64, 9], [8, 8], [1, 8]],
            )
            nc.vector.tensor_copy(dst_ap, src_ap)
            # replicate partitions 0:16 -> all 128 (3 doublings)
            for step in (16, 32, 64):
                nc.sync.dma_start(idxw[step:2 * step, :], idxw[0:step, :])

        poolxp.__exit__(None, None, None)
        pool_psB.__exit__(None, None, None)
        pool_psA.__exit__(None, None, None)

        # =============== stages 5-8: gather / combine / transpose / matmul ===
        gpool = ctx.enter_context(tc.tile_pool(name="gq", bufs=4))
        mpool = ctx.enter_context(tc.tile_pool(name="mt", bufs=2))
        vpool = ctx.enter_context(tc.tile_pool(name="vt", bufs=2))
        opool = ctx.enter_context(tc.tile_pool(name="ot", bufs=2))

        def cwb2(cwd, corner, k):
            """broadcast AP over cwd half: [128, 8 ranks, 256] with inner
            step-1 pair (w, w) repeated 128x: (rank: 72, rep: 0 x128, dup: 1 x2)
            """
            base = cwd[:]
            off = base.offset + (corner * 18 + k * 2)
            return bass.AP(
                base.tensor, off,
                [list(base.ap[0]), [72, 8], [0, 128], [1, 2]],
            )

        with nc.named_scope("s58_main"), \
             tc.tile_pool(name="psV", bufs=3, space="PSUM") as psV, \
             tc.tile_pool(name="psD", bufs=1, space="PSUM") as psD:
            for h2 in range(2):
                cwd = cwds[h2]
                idxw = idxws[h2]
                ps_o = psD.tile([128, 2, 1024], F32, tag="pso", name="pso")
                for k in range(9):
                    gq = gpool.tile([128, 8, 1024], F16, tag="gq", name="gq")
                    nc.gpsimd.dma_gather(
                        gq[:],
                        xq,
                        idxw[:, k * 64: k * 64 + 64],
                        1024,
                        1024,
                        1024,
                        transpose=False,
                        queue_num=(k + h2 * 2) % 4,
                        single_packet=False,
                    )
                    # weight the 4 corner slices on DVE (2x mode via dup pairs)
                    m0 = mpool.tile([128, 8, 256], F16, tag="m0", name="m0")
                    m1 = mpool.tile([128, 8, 256], F16, tag="m1", name="m1")
                    m2 = mpool.tile([128, 8, 256], F16, tag="m2", name="m2")
                    m3 = mpool.tile([128, 8, 256], F16, tag="m3", name="m3")
                    # elem layout: [cx(2), cy(2), c(256)]; corner idx = cy*2+cx
                    nc.vector.tensor_tensor(
                        m0[:], gq[:, :, 0:256], cwb2(cwd, 0, k), Alu.mult)
                    nc.vector.tensor_tensor(
                        m2[:], gq[:, :, 256:512], cwb2(cwd, 2, k), Alu.mult)
                    nc.vector.tensor_tensor(
                        m1[:], gq[:, :, 512:768], cwb2(cwd, 1, k), Alu.mult)
                    nc.vector.tensor_tensor(
                        m3[:], gq[:, :, 768:1024], cwb2(cwd, 3, k), Alu.mult)

                    # transpose-accumulate all 4 corners to channel-major
                    valk = vpool.tile([128, 2, 1024], F16, tag="valk",
                                      name="valk")
                    for cb in range(2):
                        for rg in range(2):
                            ps_v = psV.tile([128, 512], F32, tag="psv",
                                            name="psv")
                            for rr in range(4):
                                rank = rg * 4 + rr
                                for ci, m in enumerate((m0, m1, m2, m3)):
                                    nc.tensor.matmul(
                                        ps_v[:, rr * 128:(rr + 1) * 128],
                                        m[:, rank, cb * 128:(cb + 1) * 128],
                                        eye16[:],
                                        start=(ci == 0), stop=(ci == 3),
                                    )
                            nc.scalar.copy(
                                valk[:, cb, rg * 512:(rg + 1) * 512], ps_v[:])

                    # deform matmul: accumulate over k into ps_o
                    for cb in range(2):
                        for nn in range(2):
                            nc.tensor.matmul(
                                ps_o[:, cb, nn * 512:(nn + 1) * 512],
                                wdef_sb[:, k, cb, :],
                                valk[:, cb, nn * 512:(nn + 1) * 512],
                                start=(k == 0),
                                stop=(k == 8),
                            )

                # epilogue for this h2: PSUM -> SBUF -> DRAM
                outsb = opool.tile([128, 2, 1024], F32, tag="outsb",
                                   name="outsb")
                for cb in range(2):
                    nc.scalar.copy(outsb[:, cb, :], ps_o[:, cb, :])
                    nc.sync.dma_start(
                        out_d[cb * 128:(cb + 1) * 128,
                              h2 * 1024:(h2 + 1) * 1024],
                        outsb[:, cb, :],
                    )

    nc.compile()
    return nc


def prep_inputs(x, w_off, b_off, w_deform):
    """host-side layout prep; returns list of 8 in_maps"""
    x = np.asarray(x, np.float32)
    w_off = np.asarray(w_off, np.float32)
    b_off = np.asarray(b_off, np.float32)
    w_deform = np.asarray(w_deform, np.float32)

    # weights (shared by all cores)
    # woff[ch, cb, k, oc] = w_off[oc, cb*128+ch, k//3, k%3]
    wo = w_off.reshape(18, 2, 128, 3, 3).transpose(2, 1, 3, 4, 0)  # ch, cb, ki, kj, oc
    woff = np.ascontiguousarray(wo.reshape(128, 2, 9, 18), np.float16)
    # wdef[row, k, cb, col]: block-diag of groups (2cb, 2cb+1)
    wd = np.zeros((128, 9, 2, 128), np.float32)
    wg = w_deform.reshape(4, 64, 64, 9)  # [g, o, i, k]
    for cb in range(2):
        for gg in range(2):
            g = 2 * cb + gg
            # rows gg*64..gg*64+63 = i, cols gg*64.. = o
            # target block shape [64 i, 9 k, 64 o]; wg[g] is [o, i, k]
            wd[gg * 64:(gg + 1) * 64, :, cb, gg * 64:(gg + 1) * 64] = (
                wg[g].transpose(1, 2, 0)
            )
    wdef = np.ascontiguousarray(wd, np.float16)

    eye16 = np.eye(128, dtype=np.float16)

    # per-batch quad tokens
    xqs = []
    for b in range(B):
        P4 = np.pad(x[b], ((0, 0), (1, 2), (1, 2)))  # [256, 67, 67]
        xq = np.zeros((NTOK, 2, 2, 256), np.float16)
        for cx in range(2):
            for cy in range(2):
                sub = P4[:, cy:cy + WG, cx:cx + WG]          # [256, 66, 66]
                xq[:, cx, cy, :] = sub.transpose(1, 2, 0).reshape(NTOK, 256)
        xqs.append(np.ascontiguousarray(xq.reshape(NTOK, 1024)))

    in_maps = []
    for core in range(8):
        b, half = core // 2, core % 2
        yr0 = HALF * half
        # xpad [2][128, 34, 66]: rows yr0-1..yr0+32, 1 col pad each side
        xp = np.zeros((2, 128, 34, 66), np.float16)
        r0, r1 = yr0 - 1, yr0 + 33
        sr0, sr1 = max(r0, 0), min(r1, H)
        xp[:, :, sr0 - r0:sr0 - r0 + (sr1 - sr0), 1:65] = (
            x[b].reshape(2, 128, H, W)[:, :, sr0:sr1, :]
        )
        # grids
        q = np.arange(128)[:, None, None]
        t = np.arange(16)[None, :, None]
        k = np.arange(9)[None, None, :]
        pix = t * 128 + q
        ybk = (yr0 + pix // 64 + k // 3 - 1 + b_off[2 * k]).astype(np.float32)
        xbk = (pix % 64 + k % 3 - 1 + b_off[2 * k + 1]).astype(np.float32)
        in_maps.append({
            "xpad0": np.ascontiguousarray(xp[0]),
            "xpad1": np.ascontiguousarray(xp[1]),
            "xq": xqs[b],
            "woff": woff,
            "wdef": wdef,
            "ybk": np.ascontiguousarray(ybk),
            "xbk": np.ascontiguousarray(xbk),
            "eye16": eye16,
        })
    return in_maps


def kernel(x, w_off, b_off, w_deform):
    from concourse.bass_utils import run_bass_kernel_spmd

    if "nc" not in _NC_CACHE:
        _NC_CACHE["nc"] = build_module()
    nc = _NC_CACHE["nc"]
    in_maps = prep_inputs(x, w_off, b_off, w_deform)
    res = run_bass_kernel_spmd(nc, in_maps, list(range(8)))
    out = np.zeros((B, C, H, W), np.float32)
    for core in range(8):
        b, half = core // 2, core % 2
        out[b, :, HALF * half:HALF * (half + 1), :] = (
            res.results[core]["out"].reshape(C, HALF, W)
        )
    return out
